# revision 5
# baseline (speedup 1.0000x reference)
"""Additive attention (nn_AdditiveAttention) on 8 Trainium2 NeuronCores.

Math (per batch b):
  qp = queries[b] @ W_q ; kp = keys[b] @ W_k        # (L, H)
  S[q,k] = sum_h w_v[h] * tanh(qp[q,h] + kp[k,h])
  out[b] = softmax_k(S, masked to k < valid_lens[b]) @ values[b]

Key trick: tanh(x) ~= sum_{m=1..M} a_m sin(m*w*x), and
sin(mw(qp+kp)) = sin(mw qp)cos(mw kp) + cos(mw qp)sin(mw kp), so S is
ONE PE matmul contraction of depth 2*M*H over separable sin/cos
factors -- no O(Lq*Lk*H) elementwise work.

Engine mapping (per measured TRN2 op costs):
  - projections: PE fp16, one PSUM tile per (slot, hb) = [qp | kp].
  - sin/cos m=1,2 on ScalarE straight from PSUM (HW Sin window ~+-3.55
    rad; m=2 args exceed it with ~1e-4 probability, err negligible).
    Issue order c1,c2,s1,s2 so DVE work starts early.
  - m=3..6 via step-2 Chebyshev x_m = C4*x_{m-2} -+ x_{m-4} with
    C4 = 2cos(2wx): depth-2 dependency tree of fp16 DVE tensor_tensor
    2x ops on slot-fused [128,1712] tiles.
  - w_v folded into the k-part of the seeds (recurrence is linear =>
    propagates w_v free); a_m applied per m into k-only tiles, spread
    over Pool (m=1,2), ScalarE copy (m=3,4), DVE (m=5,6) so no engine
    bottlenecks.
  - scores: per (slot, kb) one 4M-matmul PSUM accumulation group,
    k stationary / q moving fp16 -> PSUM [k, q] fp32, issued
    m-ascending so PE overlaps the recurrence.
  - exp on ScalarE (bias -4 keeps fp16 range), masked num|den matmul
    against fp16 [values | mask], DVE reciprocal + scale, one DMA out.
  - host packs inputs so each tensor is ONE DMA (issue cost ~600ns
    each dominated the old front end).

SPMD: one NEFF on 8 cores, 2 slots (one batch per core per slot).
Host picks slot K = max valid_len of that slot's batches (sorted).
"""

import sys

if "/opt/trn_rl_repo" not in sys.path:
    sys.path.insert(0, "/opt/trn_rl_repo")

import numpy as np

import concourse.bacc as bacc
import concourse.mybir as mybir
import concourse.tile as tile
from concourse.bass_utils import run_bass_kernel_spmd

N_CORES = 8
B, LQ, LK = 16, 256, 256
D = 256
H = 256
DV = 256
F32 = mybir.dt.float32
F16 = mybir.dt.float16

M = 6
OMEGA = float(2 * np.pi / 13.0)
COEF = (1.129788026233727, 0.06067432087357483, 0.11426710446271948,
        0.0913172138010146, -0.0284372258474513, 0.038004127140723716)
EXP_BIAS = -4.0
# which engine applies a_m to the k-side tiles, per m (1-based)
KSC_ENG = {1: "pool", 2: "pool", 3: "act", 4: "act", 5: "dve", 6: "dve"}

Alu = mybir.AluOpType
ActF = mybir.ActivationFunctionType


def _plan(valid_lens):
    pieces = sorted(range(B), key=lambda b: -int(valid_lens[b]))
    slots = []
    for s in range(B // N_CORES):
        grp = pieces[s * N_CORES:(s + 1) * N_CORES]
        K = max(int(valid_lens[b]) for b in grp)
        K = min(LK, (K + 3) // 4 * 4)
        slots.append((K, grp))
    return slots


def _build(slot_ks):
    nc = bacc.Bacc("TRN2", target_bir_lowering=False, debug=False,
                   num_devices=N_CORES)
    # packed weights: [wq(4x128) | wk(4x128)] fp16, block (d,hb) major
    wqk_ext = nc.dram_tensor("Wqk", [128, 1024], F16,
                             kind="ExternalInput").ap()
    wv_ext = nc.dram_tensor("wv", [128, 2], F32, kind="ExternalInput").ap()
    slot_ios = []
    for su, K in enumerate(slot_ks):
        C = LQ + K
        KB = (K + 127) // 128
        slot_ios.append((
            nc.dram_tensor(f"qkT{su}", [128, 2 * C], F16,
                           kind="ExternalInput").ap(),
            nc.dram_tensor(f"vx{su}", [128, KB * (DV + 1)], F16,
                           kind="ExternalInput").ap(),
            nc.dram_tensor(f"out{su}", [128, 2 * DV], F32,
                           kind="ExternalOutput").ap(),
        ))

    offs = []
    o = 0
    for su, K in enumerate(slot_ks):
        for hb in range(2):
            offs.append(o)
            o += LQ + K
    CT = o
    koffs = []
    o = 0
    for su, K in enumerate(slot_ks):
        for hb in range(2):
            koffs.append(o)
            o += K
    KTOT = o

    def frag(su, hb):
        return offs[su * 2 + hb]

    def kfrag(su, hb):
        return koffs[su * 2 + hb]

    with tile.TileContext(nc) as tc:
        with (
            tc.tile_pool(name="consts", bufs=1) as consts,
            tc.tile_pool(name="io", bufs=1) as iop,
            tc.tile_pool(name="sb", bufs=1) as sb,
            tc.tile_pool(name="post", bufs=2) as postp,
            tc.tile_pool(name="pps", bufs=1, space="PSUM") as proj_ps,
            tc.tile_pool(name="sps", bufs=2, space="PSUM") as sc_psp,
            tc.tile_pool(name="aps", bufs=2, space="PSUM") as av_psp,
        ):
            wqk_t = consts.tile([128, 1024], F16, tag="wqk", name="wqk")
            nc.sync.dma_start(wqk_t[:], wqk_ext)
            wv_t = consts.tile([128, 2], F32, tag="wv", name="wv")
            nc.sync.dma_start(wv_t[:], wv_ext)

            def wq(d, hb):
                return wqk_t[:, d * 256 + hb * 128:d * 256 + hb * 128 + 128]

            def wk(d, hb):
                return wqk_t[:, 512 + d * 256 + hb * 128:
                             512 + d * 256 + hb * 128 + 128]

            qkT_t = []
            vx_t = []
            for su, K in enumerate(slot_ks):
                C = LQ + K
                KB = (K + 127) // 128
                qkT_ext, vx_ext, _ = slot_ios[su]
                t = iop.tile([128, 2 * C], F16, tag=f"qkT{su}",
                             name=f"qkT{su}")
                qkT_t.append(t)
                nc.sync.dma_start(t[:], qkT_ext)
                vt = iop.tile([128, KB * (DV + 1)], F16, tag=f"vx{su}",
                              name=f"vx{su}")
                vx_t.append(vt)
                nc.sync.dma_start(vt[:], vx_ext)
            hp = consts.tile([128, 1], F32, tag="hp", name="hp")
            nc.vector.memset(hp[:], float(np.pi / 2))
            nb = consts.tile([128, 1], F32, tag="nb", name="nb")
            nc.vector.memset(nb[:], EXP_BIAS)

            # ---- projections: psum[su,hb] = [qp(256) | kp(K)] ----
            pj = []
            for su, K in enumerate(slot_ks):
                C = LQ + K
                for hb in range(2):
                    p = proj_ps.tile([128, C], F32,
                                     tag=f"pj{su}{hb}", name=f"pj{su}{hb}")
                    pj.append(p)
                    nc.tensor.matmul(p[:, 0:LQ], wq(0, hb),
                                     qkT_t[su][:, 0:LQ],
                                     start=True, stop=False,
                                     skip_group_check=True)
                    nc.tensor.matmul(p[:, 0:LQ], wq(1, hb),
                                     qkT_t[su][:, C:C + LQ],
                                     start=False, stop=False,
                                     skip_group_check=True)
                    nc.tensor.matmul(p[:, LQ:C], wk(0, hb),
                                     qkT_t[su][:, LQ:C],
                                     start=False, stop=False,
                                     skip_group_check=True)
                    nc.tensor.matmul(p[:, LQ:C], wk(1, hb),
                                     qkT_t[su][:, C + LQ:2 * C],
                                     start=False, stop=True,
                                     skip_group_check=True)

            # ---- sin/cos tiles [128, CT]; sct[m-1][0]=sin, [1]=cos ----
            sct = [[sb.tile([128, CT], F16, tag=f"s{m}_{j}", name=f"s{m}_{j}")
                    for j in range(2)] for m in range(M)]
            # c1, c2 first so DVE/Pool work can start early
            for (m, j) in ((1, 1), (2, 1), (1, 0), (2, 0)):
                for su, K in enumerate(slot_ks):
                    for hb in range(2):
                        p = pj[su * 2 + hb]
                        f = frag(su, hb)
                        bias = hp[:] if j else 0.0
                        nc.scalar.activation(
                            sct[m - 1][j][:, f:f + LQ + K],
                            p[:], ActF.Sin, bias=bias, scale=m * OMEGA)

            # C4 = 2*cos(2wx) from raw c2; c4 = 2*c2_raw^2 - 1 (raw)
            c4t = sb.tile([128, CT], F16, tag="c4t", name="c4t")
            nc.vector.tensor_scalar_mul(c4t[:], sct[1][1][:], 2.0)
            u4 = sb.tile([128, CT], F16, tag="u4", name="u4")
            nc.vector.tensor_tensor(u4[:], c4t[:], sct[1][1][:], Alu.mult)
            nc.vector.tensor_scalar_sub(sct[3][1][:], u4[:], 1.0)

            # fold w_v into the k-parts of the seeds (in place):
            # c1, c2, s1, s2 and the raw c4 computed above.
            for (m, j) in ((1, 1), (2, 1), (4, 1), (1, 0), (2, 0)):
                for su, K in enumerate(slot_ks):
                    for hb in range(2):
                        ko = frag(su, hb) + LQ
                        nc.vector.tensor_scalar_mul(
                            sct[m - 1][j][:, ko:ko + K],
                            sct[m - 1][j][:, ko:ko + K],
                            wv_t[:, hb:hb + 1])

            # ---- ksc[m][j]: a_m * k-part, layout [128, KTOT] ----
            ksc = [[sb.tile([128, KTOT], F16, tag=f"k{m}_{j}",
                            name=f"k{m}_{j}") for j in range(2)]
                   for m in range(M)]

            def emit_ksc(m):
                eng = KSC_ENG[m]
                for j in range(2):
                    for su, K in enumerate(slot_ks):
                        for hb in range(2):
                            src = sct[m - 1][j][:, frag(su, hb) + LQ:
                                                frag(su, hb) + LQ + K]
                            dst = ksc[m - 1][j][:, kfrag(su, hb):
                                                kfrag(su, hb) + K]
                            if eng == "act":
                                nc.scalar.activation(dst, src, ActF.Copy,
                                                     scale=float(COEF[m - 1]))
                            elif eng == "pool":
                                nc.gpsimd.tensor_scalar_mul(
                                    dst, src, float(COEF[m - 1]))
                            else:
                                nc.vector.tensor_scalar_mul(
                                    dst, src, float(COEF[m - 1]))

            emit_ksc(1)
            emit_ksc(2)

            # ---- step-2 Chebyshev: x_m = C4*x_{m-2} -+ x_{m-4} ----
            tm = [sb.tile([128, CT], F16, tag=f"tm{i}", name=f"tm{i}")
                  for i in range(2)]
            # level 1: m=3 (from m=1), m=4 sin (c4 done above)
            nc.vector.tensor_tensor(tm[0][:], c4t[:], sct[0][1][:], Alu.mult)
            nc.vector.tensor_tensor(sct[2][1][:], tm[0][:], sct[0][1][:],
                                    Alu.subtract)
            nc.vector.tensor_tensor(tm[1][:], c4t[:], sct[0][0][:], Alu.mult)
            nc.vector.tensor_tensor(sct[2][0][:], tm[1][:], sct[0][0][:],
                                    Alu.add)
            nc.vector.tensor_tensor(sct[3][0][:], c4t[:], sct[1][0][:],
                                    Alu.mult)
            emit_ksc(3)
            emit_ksc(4)
            # level 2: m=5 (from m=3, m=1), m=6 (from m=4, m=2)
            if M >= 5:
                nc.vector.tensor_tensor(tm[0][:], c4t[:], sct[2][1][:],
                                        Alu.mult)
                nc.vector.tensor_tensor(sct[4][1][:], tm[0][:], sct[0][1][:],
                                        Alu.subtract)
                nc.vector.tensor_tensor(tm[1][:], c4t[:], sct[2][0][:],
                                        Alu.mult)
                nc.vector.tensor_tensor(sct[4][0][:], tm[1][:], sct[0][0][:],
                                        Alu.subtract)
                emit_ksc(5)
            if M >= 6:
                nc.vector.tensor_tensor(tm[0][:], c4t[:], sct[3][1][:],
                                        Alu.mult)
                nc.vector.tensor_tensor(sct[5][1][:], tm[0][:], sct[1][1][:],
                                        Alu.subtract)
                nc.vector.tensor_tensor(tm[1][:], c4t[:], sct[3][0][:],
                                        Alu.mult)
                nc.vector.tensor_tensor(sct[5][0][:], tm[1][:], sct[1][0][:],
                                        Alu.subtract)
                emit_ksc(6)

            # ---- scores -> exp -> attn@values -> out ----
            for su, K in enumerate(slot_ks):
                _, _, out_ext = slot_ios[su]
                KB = (K + 127) // 128
                expT = [sb.tile([128, LQ], F16, tag=f"e{su}_{kb}",
                                name=f"e{su}_{kb}") for kb in range(KB)]
                for kb in range(KB):
                    kr = min(128, K - kb * 128)
                    scp = sc_psp.tile([128, LQ], F32, tag="sc", name="sc")
                    n_mm = 4 * M
                    i = 0
                    for m in range(M):
                        for j in range(2):
                            for hb in range(2):
                                ko = kfrag(su, hb) + kb * 128
                                qo = frag(su, hb)
                                nc.tensor.matmul(
                                    scp[:kr, :],
                                    ksc[m][1 - j][:, ko:ko + kr],
                                    sct[m][j][:, qo:qo + LQ],
                                    start=(i == 0), stop=(i == n_mm - 1))
                                i += 1
                    nc.scalar.activation(expT[kb][:kr, :], scp[:kr, :],
                                         ActF.Exp, bias=nb[:kr, :])
                ot = postp.tile([128, 2 * DV], F32, tag=f"ot{su}",
                                name=f"ot{su}", bufs=1)
                for qb in range(LQ // 128):
                    av = av_psp.tile([128, DV + 1], F32, tag="av", name="av")
                    for kb in range(KB):
                        kr = min(128, K - kb * 128)
                        nc.tensor.matmul(
                            av[:, :],
                            expT[kb][:kr, qb * 128:(qb + 1) * 128],
                            vx_t[su][:kr, kb * (DV + 1):
                                     kb * (DV + 1) + DV + 1],
                            start=(kb == 0), stop=(kb == KB - 1))
                    rec = postp.tile([128, 1], F32, tag="rec", name="rec")
                    nc.vector.reciprocal(rec[:], av[:, DV:DV + 1])
                    nc.vector.tensor_scalar_mul(
                        ot[:, qb * DV:(qb + 1) * DV], av[:, 0:DV], rec[:])
                nc.sync.dma_start(out_ext, ot[:])
    nc.compile()
    return nc


_CACHE = {}


def _get_graph(slot_ks):
    key = tuple(slot_ks)
    if key not in _CACHE:
        _CACHE[key] = _build(slot_ks)
    return _CACHE[key]


def _build_in_maps(queries, keys, values, valid_lens, W_q, W_k, w_v, slots):
    # packed weights: block (d, hb) major, wq then wk
    wqk = np.empty((128, 1024), np.float16)
    for d in range(2):
        for hb in range(2):
            wqk[:, d * 256 + hb * 128:d * 256 + hb * 128 + 128] = \
                W_q[d * 128:(d + 1) * 128,
                    hb * 128:(hb + 1) * 128].astype(np.float16)
            wqk[:, 512 + d * 256 + hb * 128:512 + d * 256 + hb * 128 + 128] \
                = W_k[d * 128:(d + 1) * 128,
                      hb * 128:(hb + 1) * 128].astype(np.float16)
    wvs = np.ascontiguousarray(w_v.astype(np.float32).reshape(2, 128).T)
    in_maps = [{"Wqk": wqk, "wv": wvs} for _ in range(N_CORES)]
    for su, (K, grp) in enumerate(slots):
        C = LQ + K
        KB = (K + 127) // 128
        for c, b in enumerate(grp):
            vl = int(valid_lens[b])
            qT = queries[b].T.astype(np.float16)          # [D, LQ]
            kT = keys[b, :K, :].T.astype(np.float16)      # [D, K]
            qk = np.empty((128, 2 * C), np.float16)
            for d in range(2):
                qk[:, d * C:d * C + LQ] = qT[d * 128:(d + 1) * 128]
                qk[:, d * C + LQ:(d + 1) * C] = kT[d * 128:(d + 1) * 128]
            in_maps[c][f"qkT{su}"] = qk
            vpad = np.zeros((KB * 128, DV + 1), np.float16)
            vpad[:vl, :DV] = values[b, :vl, :].astype(np.float16)
            vpad[:vl, DV] = 1.0
            in_maps[c][f"vx{su}"] = np.ascontiguousarray(
                vpad.reshape(KB, 128, DV + 1).transpose(1, 0, 2)
                .reshape(128, KB * (DV + 1)))
    return in_maps


def kernel(queries, keys, values, valid_lens, W_q, W_k, w_v):
    queries = np.asarray(queries, dtype=np.float32)
    keys = np.asarray(keys, dtype=np.float32)
    values = np.asarray(values, dtype=np.float32)
    valid_lens = np.asarray(valid_lens)
    W_q = np.asarray(W_q, dtype=np.float32)
    W_k = np.asarray(W_k, dtype=np.float32)
    w_v = np.asarray(w_v, dtype=np.float32)

    slots = _plan(valid_lens)
    nc = _get_graph([K for (K, _) in slots])
    in_maps = _build_in_maps(queries, keys, values, valid_lens,
                             W_q, W_k, w_v, slots)
    res = run_bass_kernel_spmd(nc, in_maps, list(range(N_CORES)))

    out = np.empty((B, LQ, DV), np.float32)
    for su, (K, grp) in enumerate(slots):
        for c, b in enumerate(grp):
            o = res.results[c][f"out{su}"]        # [128, 2*DV]
            out[b] = o.reshape(128, 2, DV).transpose(1, 0, 2).reshape(LQ, DV)
    return out


# revision 6
# speedup vs baseline: 1.5470x; 1.5470x over previous
"""Additive attention (nn_AdditiveAttention) on 8 Trainium2 NeuronCores.

Math (per batch b):
  qp = queries[b] @ W_q ; kp = keys[b] @ W_k        # (L, H)
  S[q,k] = sum_h w_v[h] * tanh(qp[q,h] + kp[k,h])
  out[b] = softmax_k(S, masked to k < valid_lens[b]) @ values[b]

Key trick: tanh(x) ~= sum_{m=1..M} a_m sin(m*w*x), and
sin(mw(qp+kp)) = sin(mw qp)cos(mw kp) + cos(mw qp)sin(mw kp), so S is
ONE PE matmul contraction of depth 2*M*H over separable sin/cos
factors -- no O(Lq*Lk*H) elementwise work.

Engine mapping (per measured TRN2 op costs):
  - projections: PE fp16, one PSUM tile per (slot, hb) = [qp | kp].
  - sin/cos m=1,2 on ScalarE straight from PSUM (HW Sin window ~+-3.55
    rad; m=2 args exceed it with ~1e-4 probability, err negligible).
    Issue order c1,c2,s1,s2 so DVE work starts early.
  - m=3..6 via step-2 Chebyshev x_m = C4*x_{m-2} -+ x_{m-4} with
    C4 = 2cos(2wx): depth-2 dependency tree of fp16 DVE tensor_tensor
    2x ops on slot-fused [128,1712] tiles.
  - w_v folded into the k-part of the seeds (recurrence is linear =>
    propagates w_v free); a_m applied per m into k-only tiles, spread
    over Pool (m=1,2), ScalarE copy (m=3,4), DVE (m=5,6) so no engine
    bottlenecks.
  - scores: per (slot, kb) one 4M-matmul PSUM accumulation group,
    k stationary / q moving fp16 -> PSUM [k, q] fp32, issued
    m-ascending so PE overlaps the recurrence.
  - exp on ScalarE (bias -4 keeps fp16 range), masked num|den matmul
    against fp16 [values | mask], DVE reciprocal + scale, one DMA out.
  - host packs inputs so each tensor is ONE DMA (issue cost ~600ns
    each dominated the old front end).

SPMD: one NEFF on 8 cores, 2 slots (one batch per core per slot).
Host picks slot K = max valid_len of that slot's batches (sorted).
"""

import sys

if "/opt/trn_rl_repo" not in sys.path:
    sys.path.insert(0, "/opt/trn_rl_repo")

import numpy as np

import concourse.bacc as bacc
import concourse.mybir as mybir
import concourse.tile as tile
from concourse.bass_utils import run_bass_kernel_spmd

N_CORES = 8
B, LQ, LK = 16, 256, 256
D = 256
H = 256
DV = 256
F32 = mybir.dt.float32
F16 = mybir.dt.float16

M = 6
OMEGA = float(2 * np.pi / 13.0)
COEF = (1.129788026233727, 0.06067432087357483, 0.11426710446271948,
        0.0913172138010146, -0.0284372258474513, 0.038004127140723716)
EXP_BIAS = -4.0
# which engine applies a_m to the k-side tiles, per m (1-based)
KSC_ENG = {1: "act", 2: "act", 3: "act", 4: "dve", 5: "dve", 6: "dve"}

Alu = mybir.AluOpType
ActF = mybir.ActivationFunctionType


def _plan(valid_lens):
    pieces = sorted(range(B), key=lambda b: -int(valid_lens[b]))
    slots = []
    for s in range(B // N_CORES):
        grp = pieces[s * N_CORES:(s + 1) * N_CORES]
        K = max(int(valid_lens[b]) for b in grp)
        K = min(LK, (K + 3) // 4 * 4)
        slots.append((K, grp))
    return slots


def _build(slot_ks):
    nc = bacc.Bacc("TRN2", target_bir_lowering=False, debug=False,
                   num_devices=N_CORES)
    # packed weights: [wq(4x128) | wk(4x128)] fp16, block (d,hb) major
    wqk_ext = nc.dram_tensor("Wqk", [128, 1024], F16,
                             kind="ExternalInput").ap()
    wv_ext = nc.dram_tensor("wv", [128, 2], F32, kind="ExternalInput").ap()
    slot_ios = []
    for su, K in enumerate(slot_ks):
        C = LQ + K
        KB = (K + 127) // 128
        slot_ios.append((
            nc.dram_tensor(f"qkT{su}", [128, 2 * C], F16,
                           kind="ExternalInput").ap(),
            nc.dram_tensor(f"vx{su}", [128, KB * (DV + 1)], F16,
                           kind="ExternalInput").ap(),
            nc.dram_tensor(f"out{su}", [128, 2 * DV], F32,
                           kind="ExternalOutput").ap(),
        ))

    offs = []
    o = 0
    for su, K in enumerate(slot_ks):
        for hb in range(2):
            offs.append(o)
            o += LQ + K
    CT = o
    koffs = []
    o = 0
    for su, K in enumerate(slot_ks):
        for hb in range(2):
            koffs.append(o)
            o += K
    KTOT = o

    def frag(su, hb):
        return offs[su * 2 + hb]

    def kfrag(su, hb):
        return koffs[su * 2 + hb]

    with tile.TileContext(nc) as tc:
        with (
            tc.tile_pool(name="consts", bufs=1) as consts,
            tc.tile_pool(name="io", bufs=1) as iop,
            tc.tile_pool(name="sb", bufs=1) as sb,
            tc.tile_pool(name="post", bufs=2) as postp,
            tc.tile_pool(name="pps", bufs=1, space="PSUM") as proj_ps,
            tc.tile_pool(name="sps", bufs=2, space="PSUM") as sc_psp,
            tc.tile_pool(name="aps", bufs=2, space="PSUM") as av_psp,
        ):
            wqk_t = consts.tile([128, 1024], F16, tag="wqk", name="wqk")
            nc.sync.dma_start(wqk_t[:], wqk_ext)
            wv_t = consts.tile([128, 2], F32, tag="wv", name="wv")
            nc.sync.dma_start(wv_t[:], wv_ext)

            def wq(d, hb):
                return wqk_t[:, d * 256 + hb * 128:d * 256 + hb * 128 + 128]

            def wk(d, hb):
                return wqk_t[:, 512 + d * 256 + hb * 128:
                             512 + d * 256 + hb * 128 + 128]

            qkT_t = []
            vx_t = []
            for su, K in enumerate(slot_ks):
                C = LQ + K
                KB = (K + 127) // 128
                qkT_ext, vx_ext, _ = slot_ios[su]
                t = iop.tile([128, 2 * C], F16, tag=f"qkT{su}",
                             name=f"qkT{su}")
                qkT_t.append(t)
                nc.sync.dma_start(t[:], qkT_ext)
                vt = iop.tile([128, KB * (DV + 1)], F16, tag=f"vx{su}",
                              name=f"vx{su}")
                vx_t.append(vt)
                nc.sync.dma_start(vt[:], vx_ext)
            hp = consts.tile([128, 1], F32, tag="hp", name="hp")
            nc.vector.memset(hp[:], float(np.pi / 2))
            nb = consts.tile([128, 1], F32, tag="nb", name="nb")
            nc.vector.memset(nb[:], EXP_BIAS)

            # ---- projections: psum[su,hb] = [qp(256) | kp(K)] ----
            pj = []
            for su, K in enumerate(slot_ks):
                C = LQ + K
                for hb in range(2):
                    p = proj_ps.tile([128, C], F32,
                                     tag=f"pj{su}{hb}", name=f"pj{su}{hb}")
                    pj.append(p)
                    nc.tensor.matmul(p[:, 0:LQ], wq(0, hb),
                                     qkT_t[su][:, 0:LQ],
                                     start=True, stop=False,
                                     skip_group_check=True)
                    nc.tensor.matmul(p[:, 0:LQ], wq(1, hb),
                                     qkT_t[su][:, C:C + LQ],
                                     start=False, stop=False,
                                     skip_group_check=True)
                    nc.tensor.matmul(p[:, LQ:C], wk(0, hb),
                                     qkT_t[su][:, LQ:C],
                                     start=False, stop=False,
                                     skip_group_check=True)
                    nc.tensor.matmul(p[:, LQ:C], wk(1, hb),
                                     qkT_t[su][:, C + LQ:2 * C],
                                     start=False, stop=True,
                                     skip_group_check=True)

            # ---- sin/cos tiles [128, CT]; sct[m-1][0]=sin, [1]=cos ----
            sct = [[sb.tile([128, CT], F16, tag=f"s{m}_{j}", name=f"s{m}_{j}")
                    for j in range(2)] for m in range(M)]
            # c1, c2 first so DVE/Pool work can start early
            for (m, j) in ((1, 1), (2, 1), (1, 0), (2, 0)):
                for su, K in enumerate(slot_ks):
                    for hb in range(2):
                        p = pj[su * 2 + hb]
                        f = frag(su, hb)
                        bias = hp[:] if j else 0.0
                        nc.scalar.activation(
                            sct[m - 1][j][:, f:f + LQ + K],
                            p[:], ActF.Sin, bias=bias, scale=m * OMEGA)

            # C4 = 2*cos(2wx) from raw c2; c4 = 2*c2_raw^2 - 1 (raw)
            c4t = sb.tile([128, CT], F16, tag="c4t", name="c4t")
            nc.vector.tensor_scalar_mul(c4t[:], sct[1][1][:], 2.0)
            u4 = sb.tile([128, CT], F16, tag="u4", name="u4")
            nc.vector.tensor_tensor(u4[:], c4t[:], sct[1][1][:], Alu.mult)
            nc.vector.tensor_scalar_sub(sct[3][1][:], u4[:], 1.0)

            # fold w_v into the k-parts of the seeds (in place):
            # c1, c2, s1, s2 and the raw c4 computed above.
            for (m, j) in ((1, 1), (2, 1), (4, 1), (1, 0), (2, 0)):
                for su, K in enumerate(slot_ks):
                    for hb in range(2):
                        ko = frag(su, hb) + LQ
                        nc.vector.tensor_scalar_mul(
                            sct[m - 1][j][:, ko:ko + K],
                            sct[m - 1][j][:, ko:ko + K],
                            wv_t[:, hb:hb + 1])

            # ---- ksc[m][j]: a_m * k-part, layout [128, KTOT] ----
            ksc = [[sb.tile([128, KTOT], F16, tag=f"k{m}_{j}",
                            name=f"k{m}_{j}") for j in range(2)]
                   for m in range(M)]

            def emit_ksc(m):
                eng = KSC_ENG[m]
                for j in range(2):
                    for su, K in enumerate(slot_ks):
                        for hb in range(2):
                            src = sct[m - 1][j][:, frag(su, hb) + LQ:
                                                frag(su, hb) + LQ + K]
                            dst = ksc[m - 1][j][:, kfrag(su, hb):
                                                kfrag(su, hb) + K]
                            if eng == "act":
                                nc.scalar.activation(dst, src, ActF.Copy,
                                                     scale=float(COEF[m - 1]))
                            elif eng == "pool":
                                nc.gpsimd.tensor_scalar_mul(
                                    dst, src, float(COEF[m - 1]))
                            else:
                                nc.vector.tensor_scalar_mul(
                                    dst, src, float(COEF[m - 1]))

            emit_ksc(1)
            emit_ksc(2)

            # ---- step-2 Chebyshev: x_m = C4*x_{m-2} -+ x_{m-4} ----
            tm = [sb.tile([128, CT], F16, tag=f"tm{i}", name=f"tm{i}")
                  for i in range(2)]
            # level 1: m=3 (from m=1), m=4 sin (c4 done above)
            nc.vector.tensor_tensor(tm[0][:], c4t[:], sct[0][1][:], Alu.mult)
            nc.vector.tensor_tensor(sct[2][1][:], tm[0][:], sct[0][1][:],
                                    Alu.subtract)
            nc.vector.tensor_tensor(tm[1][:], c4t[:], sct[0][0][:], Alu.mult)
            nc.vector.tensor_tensor(sct[2][0][:], tm[1][:], sct[0][0][:],
                                    Alu.add)
            nc.vector.tensor_tensor(sct[3][0][:], c4t[:], sct[1][0][:],
                                    Alu.mult)
            emit_ksc(3)
            emit_ksc(4)
            # level 2: m=5 (from m=3, m=1), m=6 (from m=4, m=2)
            if M >= 5:
                nc.vector.tensor_tensor(tm[0][:], c4t[:], sct[2][1][:],
                                        Alu.mult)
                nc.vector.tensor_tensor(sct[4][1][:], tm[0][:], sct[0][1][:],
                                        Alu.subtract)
                nc.vector.tensor_tensor(tm[1][:], c4t[:], sct[2][0][:],
                                        Alu.mult)
                nc.vector.tensor_tensor(sct[4][0][:], tm[1][:], sct[0][0][:],
                                        Alu.subtract)
                emit_ksc(5)
            if M >= 6:
                nc.vector.tensor_tensor(tm[0][:], c4t[:], sct[3][1][:],
                                        Alu.mult)
                nc.vector.tensor_tensor(sct[5][1][:], tm[0][:], sct[1][1][:],
                                        Alu.subtract)
                nc.vector.tensor_tensor(tm[1][:], c4t[:], sct[3][0][:],
                                        Alu.mult)
                nc.vector.tensor_tensor(sct[5][0][:], tm[1][:], sct[1][0][:],
                                        Alu.subtract)
                emit_ksc(6)

            # ---- scores -> exp -> attn@values -> out ----
            for su, K in enumerate(slot_ks):
                _, _, out_ext = slot_ios[su]
                KB = (K + 127) // 128
                expT = [sb.tile([128, LQ], F16, tag=f"e{su}_{kb}",
                                name=f"e{su}_{kb}") for kb in range(KB)]
                for kb in range(KB):
                    kr = min(128, K - kb * 128)
                    scp = sc_psp.tile([128, LQ], F32, tag="sc", name="sc")
                    n_mm = 4 * M
                    i = 0
                    for m in range(M):
                        for j in range(2):
                            for hb in range(2):
                                ko = kfrag(su, hb) + kb * 128
                                qo = frag(su, hb)
                                nc.tensor.matmul(
                                    scp[:kr, :],
                                    ksc[m][1 - j][:, ko:ko + kr],
                                    sct[m][j][:, qo:qo + LQ],
                                    start=(i == 0), stop=(i == n_mm - 1))
                                i += 1
                    nc.scalar.activation(expT[kb][:kr, :], scp[:kr, :],
                                         ActF.Exp, bias=nb[:kr, :])
                ot = postp.tile([128, 2 * DV], F32, tag=f"ot{su}",
                                name=f"ot{su}", bufs=1)
                for qb in range(LQ // 128):
                    av = av_psp.tile([128, DV + 1], F32, tag="av", name="av")
                    for kb in range(KB):
                        kr = min(128, K - kb * 128)
                        nc.tensor.matmul(
                            av[:, :],
                            expT[kb][:kr, qb * 128:(qb + 1) * 128],
                            vx_t[su][:kr, kb * (DV + 1):
                                     kb * (DV + 1) + DV + 1],
                            start=(kb == 0), stop=(kb == KB - 1))
                    rec = postp.tile([128, 1], F32, tag="rec", name="rec")
                    nc.vector.reciprocal(rec[:], av[:, DV:DV + 1])
                    nc.vector.tensor_scalar_mul(
                        ot[:, qb * DV:(qb + 1) * DV], av[:, 0:DV], rec[:])
                nc.sync.dma_start(out_ext, ot[:])
    nc.compile()
    return nc


_CACHE = {}


def _get_graph(slot_ks):
    key = tuple(slot_ks)
    if key not in _CACHE:
        _CACHE[key] = _build(slot_ks)
    return _CACHE[key]


def _build_in_maps(queries, keys, values, valid_lens, W_q, W_k, w_v, slots):
    # packed weights: block (d, hb) major, wq then wk
    wqk = np.empty((128, 1024), np.float16)
    for d in range(2):
        for hb in range(2):
            wqk[:, d * 256 + hb * 128:d * 256 + hb * 128 + 128] = \
                W_q[d * 128:(d + 1) * 128,
                    hb * 128:(hb + 1) * 128].astype(np.float16)
            wqk[:, 512 + d * 256 + hb * 128:512 + d * 256 + hb * 128 + 128] \
                = W_k[d * 128:(d + 1) * 128,
                      hb * 128:(hb + 1) * 128].astype(np.float16)
    wvs = np.ascontiguousarray(w_v.astype(np.float32).reshape(2, 128).T)
    in_maps = [{"Wqk": wqk, "wv": wvs} for _ in range(N_CORES)]
    for su, (K, grp) in enumerate(slots):
        C = LQ + K
        KB = (K + 127) // 128
        for c, b in enumerate(grp):
            vl = int(valid_lens[b])
            qT = queries[b].T.astype(np.float16)          # [D, LQ]
            kT = keys[b, :K, :].T.astype(np.float16)      # [D, K]
            qk = np.empty((128, 2 * C), np.float16)
            for d in range(2):
                qk[:, d * C:d * C + LQ] = qT[d * 128:(d + 1) * 128]
                qk[:, d * C + LQ:(d + 1) * C] = kT[d * 128:(d + 1) * 128]
            in_maps[c][f"qkT{su}"] = qk
            vpad = np.zeros((KB * 128, DV + 1), np.float16)
            vpad[:vl, :DV] = values[b, :vl, :].astype(np.float16)
            vpad[:vl, DV] = 1.0
            in_maps[c][f"vx{su}"] = np.ascontiguousarray(
                vpad.reshape(KB, 128, DV + 1).transpose(1, 0, 2)
                .reshape(128, KB * (DV + 1)))
    return in_maps


def kernel(queries, keys, values, valid_lens, W_q, W_k, w_v):
    queries = np.asarray(queries, dtype=np.float32)
    keys = np.asarray(keys, dtype=np.float32)
    values = np.asarray(values, dtype=np.float32)
    valid_lens = np.asarray(valid_lens)
    W_q = np.asarray(W_q, dtype=np.float32)
    W_k = np.asarray(W_k, dtype=np.float32)
    w_v = np.asarray(w_v, dtype=np.float32)

    slots = _plan(valid_lens)
    nc = _get_graph([K for (K, _) in slots])
    in_maps = _build_in_maps(queries, keys, values, valid_lens,
                             W_q, W_k, w_v, slots)
    res = run_bass_kernel_spmd(nc, in_maps, list(range(N_CORES)))

    out = np.empty((B, LQ, DV), np.float32)
    for su, (K, grp) in enumerate(slots):
        for c, b in enumerate(grp):
            o = res.results[c][f"out{su}"]        # [128, 2*DV]
            out[b] = o.reshape(128, 2, DV).transpose(1, 0, 2).reshape(LQ, DV)
    return out


# revision 7
# speedup vs baseline: 1.6189x; 1.0465x over previous
"""Additive attention (nn_AdditiveAttention) on 8 Trainium2 NeuronCores.

Math (per batch b):
  qp = queries[b] @ W_q ; kp = keys[b] @ W_k        # (L, H)
  S[q,k] = sum_h w_v[h] * tanh(qp[q,h] + kp[k,h])
  out[b] = softmax_k(S, masked to k < valid_lens[b]) @ values[b]

Key trick: tanh(x) ~= sum_{m=1..M} a_m sin(m*w*x), and
sin(mw(qp+kp)) = sin(mw qp)cos(mw kp) + cos(mw qp)sin(mw kp), so S is
ONE PE matmul contraction of depth 2*M*H over separable sin/cos
factors -- no O(Lq*Lk*H) elementwise work.

Engine mapping (per measured TRN2 op costs):
  - projections: PE fp16, one PSUM tile per (slot, hb) = [qp | kp].
  - sin/cos m=1,2 on ScalarE straight from PSUM (HW Sin window ~+-3.55
    rad; m=2 args exceed it with ~1e-4 probability, err negligible).
    Issue order c1,c2,s1,s2 so DVE work starts early.
  - m=3..6 via step-2 Chebyshev x_m = C4*x_{m-2} -+ x_{m-4} with
    C4 = 2cos(2wx): depth-2 dependency tree of fp16 DVE tensor_tensor
    2x ops on slot-fused [128,1712] tiles.
  - w_v folded into the k-part of the seeds (recurrence is linear =>
    propagates w_v free); a_m applied per m into k-only tiles, spread
    over Pool (m=1,2), ScalarE copy (m=3,4), DVE (m=5,6) so no engine
    bottlenecks.
  - scores: per (slot, kb) one 4M-matmul PSUM accumulation group,
    k stationary / q moving fp16 -> PSUM [k, q] fp32, issued
    m-ascending so PE overlaps the recurrence.
  - exp on ScalarE (bias -4 keeps fp16 range), masked num|den matmul
    against fp16 [values | mask], DVE reciprocal + scale, one DMA out.
  - host packs inputs so each tensor is ONE DMA (issue cost ~600ns
    each dominated the old front end).

SPMD: one NEFF on 8 cores, 2 slots (one batch per core per slot).
Host picks slot K = max valid_len of that slot's batches (sorted).
"""

import sys

if "/opt/trn_rl_repo" not in sys.path:
    sys.path.insert(0, "/opt/trn_rl_repo")

import numpy as np

import concourse.bacc as bacc
import concourse.mybir as mybir
import concourse.tile as tile
from concourse.bass_utils import run_bass_kernel_spmd

N_CORES = 8
B, LQ, LK = 16, 256, 256
D = 256
H = 256
DV = 256
F32 = mybir.dt.float32
F16 = mybir.dt.float16

M = 6
OMEGA = float(2 * np.pi / 13.0)
COEF = (1.129788026233727, 0.06067432087357483, 0.11426710446271948,
        0.0913172138010146, -0.0284372258474513, 0.038004127140723716)
EXP_BIAS = -4.0
# which engine applies a_m to the k-side tiles, per m (1-based)
KSC_ENG = {2: "act", 3: "act", 4: "dve", 5: "dve", 6: "dve"}

Alu = mybir.AluOpType
ActF = mybir.ActivationFunctionType


def _plan(valid_lens):
    pieces = sorted(range(B), key=lambda b: -int(valid_lens[b]))
    slots = []
    for s in range(B // N_CORES):
        grp = pieces[s * N_CORES:(s + 1) * N_CORES]
        K = max(int(valid_lens[b]) for b in grp)
        K = min(LK, (K + 3) // 4 * 4)
        slots.append((K, grp))
    return slots


def _build(slot_ks):
    nc = bacc.Bacc("TRN2", target_bir_lowering=False, debug=False,
                   num_devices=N_CORES)
    # packed weights: [wq(4x128) | wk(4x128)] fp16, block (d,hb) major
    wqk_ext = nc.dram_tensor("Wqk", [128, 1024], F16,
                             kind="ExternalInput").ap()
    wv_ext = nc.dram_tensor("wv", [128, 2], F32, kind="ExternalInput").ap()
    slot_ios = []
    for su, K in enumerate(slot_ks):
        C = LQ + K
        KB = (K + 127) // 128
        slot_ios.append((
            nc.dram_tensor(f"qkT{su}", [128, 2 * C], F16,
                           kind="ExternalInput").ap(),
            nc.dram_tensor(f"vx{su}", [128, KB * (DV + 1)], F16,
                           kind="ExternalInput").ap(),
            nc.dram_tensor(f"out{su}", [128, 2 * DV], F32,
                           kind="ExternalOutput").ap(),
        ))

    offs = []
    o = 0
    for su, K in enumerate(slot_ks):
        for hb in range(2):
            offs.append(o)
            o += LQ + K
    CT = o
    koffs = []
    o = 0
    for su, K in enumerate(slot_ks):
        for hb in range(2):
            koffs.append(o)
            o += K
    KTOT = o

    def frag(su, hb):
        return offs[su * 2 + hb]

    def kfrag(su, hb):
        return koffs[su * 2 + hb]

    with tile.TileContext(nc) as tc:
        with (
            tc.tile_pool(name="consts", bufs=1) as consts,
            tc.tile_pool(name="io", bufs=1) as iop,
            tc.tile_pool(name="sb", bufs=1) as sb,
            tc.tile_pool(name="post", bufs=2) as postp,
            tc.tile_pool(name="pps", bufs=1, space="PSUM") as proj_ps,
            tc.tile_pool(name="sps", bufs=2, space="PSUM") as sc_psp,
            tc.tile_pool(name="aps", bufs=2, space="PSUM") as av_psp,
        ):
            wqk_t = consts.tile([128, 1024], F16, tag="wqk", name="wqk")
            nc.sync.dma_start(wqk_t[:], wqk_ext)
            wv_t = consts.tile([128, 2], F32, tag="wv", name="wv")
            nc.sync.dma_start(wv_t[:], wv_ext)

            def wq(d, hb):
                return wqk_t[:, d * 256 + hb * 128:d * 256 + hb * 128 + 128]

            def wk(d, hb):
                return wqk_t[:, 512 + d * 256 + hb * 128:
                             512 + d * 256 + hb * 128 + 128]

            qkT_t = []
            vx_t = []
            for su, K in enumerate(slot_ks):
                C = LQ + K
                KB = (K + 127) // 128
                qkT_ext, vx_ext, _ = slot_ios[su]
                t = iop.tile([128, 2 * C], F16, tag=f"qkT{su}",
                             name=f"qkT{su}")
                qkT_t.append(t)
                nc.sync.dma_start(t[:], qkT_ext)
                vt = iop.tile([128, KB * (DV + 1)], F16, tag=f"vx{su}",
                              name=f"vx{su}")
                vx_t.append(vt)
                nc.sync.dma_start(vt[:], vx_ext)
            hp = consts.tile([128, 1], F32, tag="hp", name="hp")
            nc.vector.memset(hp[:], float(np.pi / 2))
            nb = consts.tile([128, 1], F32, tag="nb", name="nb")
            nc.vector.memset(nb[:], EXP_BIAS)

            # ---- projections: psum[su,hb] = [qp(256) | kp(K)] ----
            pj = []
            for su, K in enumerate(slot_ks):
                C = LQ + K
                for hb in range(2):
                    p = proj_ps.tile([128, C], F32,
                                     tag=f"pj{su}{hb}", name=f"pj{su}{hb}")
                    pj.append(p)
                    nc.tensor.matmul(p[:, 0:LQ], wq(0, hb),
                                     qkT_t[su][:, 0:LQ],
                                     start=True, stop=False,
                                     skip_group_check=True)
                    nc.tensor.matmul(p[:, 0:LQ], wq(1, hb),
                                     qkT_t[su][:, C:C + LQ],
                                     start=False, stop=False,
                                     skip_group_check=True)
                    nc.tensor.matmul(p[:, LQ:C], wk(0, hb),
                                     qkT_t[su][:, LQ:C],
                                     start=False, stop=False,
                                     skip_group_check=True)
                    nc.tensor.matmul(p[:, LQ:C], wk(1, hb),
                                     qkT_t[su][:, C + LQ:2 * C],
                                     start=False, stop=True,
                                     skip_group_check=True)

            # ---- sin/cos tiles [128, CT]; sct[m-1][0]=sin, [1]=cos ----
            sct = [[sb.tile([128, CT], F16, tag=f"s{m}_{j}", name=f"s{m}_{j}")
                    for j in range(2)] for m in range(M)]
            # cos first so the DVE chain work can start early
            for (m, j) in ((1, 1), (2, 1), (1, 0), (2, 0)):
                for su, K in enumerate(slot_ks):
                    for hb in range(2):
                        p = pj[su * 2 + hb]
                        f = frag(su, hb)
                        bias = hp[:] if j else 0.0
                        nc.scalar.activation(
                            sct[m - 1][j][:, f:f + LQ + K],
                            p[:], ActF.Sin, bias=bias, scale=m * OMEGA)

            # ksc[m][j]: (a_m/a_1) * k-part of sct[m][j], [128, KTOT].
            # (w_v arrives pre-scaled by a_1, so m=1 needs no ksc at all:
            # the scores matmul reads sct[0] k-slices directly.)
            ksc = [None] + [[sb.tile([128, KTOT], F16, tag=f"k{m}_{j}",
                                     name=f"k{m}_{j}") for j in range(2)]
                            for m in range(1, M)]

            def emit_ksc(m, j):
                eng = KSC_ENG[m]
                r = float(COEF[m - 1] / COEF[0])
                for su, K in enumerate(slot_ks):
                    for hb in range(2):
                        src = sct[m - 1][j][:, frag(su, hb) + LQ:
                                            frag(su, hb) + LQ + K]
                        dst = ksc[m - 1][j][:, kfrag(su, hb):
                                            kfrag(su, hb) + K]
                        if eng == "act":
                            nc.scalar.activation(dst, src, ActF.Copy,
                                                 scale=r)
                        else:
                            nc.vector.tensor_scalar_mul(dst, src, r)

            def fold(m, j):
                # w_v (pre-scaled by a_1) into the k-part, in place
                for su, K in enumerate(slot_ks):
                    for hb in range(2):
                        ko = frag(su, hb) + LQ
                        nc.vector.tensor_scalar_mul(
                            sct[m - 1][j][:, ko:ko + K],
                            sct[m - 1][j][:, ko:ko + K],
                            wv_t[:, hb:hb + 1])

            tm = [sb.tile([128, CT], F16, tag=f"tm{i}", name=f"tm{i}")
                  for i in range(2)]

            def chain(dst, a, b, op):
                nc.vector.tensor_tensor(tm[0][:], c4t[:], a[:], Alu.mult)
                nc.vector.tensor_tensor(dst[:], tm[0][:], b[:], op)

            # C4 = 2*cos(2wx) and raw c4 = 2*c2_raw^2 - 1
            c4t = sb.tile([128, CT], F16, tag="c4t", name="c4t")
            nc.vector.tensor_scalar_mul(c4t[:], sct[1][1][:], 2.0)
            nc.vector.tensor_tensor(tm[1][:], c4t[:], sct[1][1][:], Alu.mult)
            nc.vector.tensor_scalar_sub(sct[3][1][:], tm[1][:], 1.0)
            # cos side: folds, chain c3/c5/c6, ksc as tiles complete
            fold(1, 1)
            fold(2, 1)
            fold(4, 1)
            emit_ksc(2, 1)
            emit_ksc(4, 1)
            chain(sct[2][1], sct[0][1], sct[0][1], Alu.subtract)   # c3
            emit_ksc(3, 1)
            chain(sct[4][1], sct[2][1], sct[0][1], Alu.subtract)   # c5
            emit_ksc(5, 1)
            chain(sct[5][1], sct[3][1], sct[1][1], Alu.subtract)   # c6
            emit_ksc(6, 1)
            # sin side
            fold(1, 0)
            fold(2, 0)
            emit_ksc(2, 0)
            chain(sct[2][0], sct[0][0], sct[0][0], Alu.add)        # s3
            emit_ksc(3, 0)
            nc.vector.tensor_tensor(sct[3][0][:], c4t[:], sct[1][0][:],
                                    Alu.mult)                      # s4
            emit_ksc(4, 0)
            chain(sct[4][0], sct[2][0], sct[0][0], Alu.subtract)   # s5
            emit_ksc(5, 0)
            chain(sct[5][0], sct[3][0], sct[1][0], Alu.subtract)   # s6
            emit_ksc(6, 0)

            # ---- scores -> exp -> attn@values -> out ----
            for su, K in enumerate(slot_ks):
                _, _, out_ext = slot_ios[su]
                KB = (K + 127) // 128
                expT = [sb.tile([128, LQ], F16, tag=f"e{su}_{kb}",
                                name=f"e{su}_{kb}") for kb in range(KB)]
                for kb in range(KB):
                    kr = min(128, K - kb * 128)
                    scp = sc_psp.tile([128, LQ], F32, tag="sc", name="sc")
                    n_mm = 4 * M
                    i = 0
                    for m in range(M):
                        for j in range(2):
                            for hb in range(2):
                                qo = frag(su, hb)
                                if m == 0:
                                    st = sct[0][1 - j][:,
                                        qo + LQ + kb * 128:
                                        qo + LQ + kb * 128 + kr]
                                else:
                                    ko = kfrag(su, hb) + kb * 128
                                    st = ksc[m][1 - j][:, ko:ko + kr]
                                nc.tensor.matmul(
                                    scp[:kr, :], st,
                                    sct[m][j][:, qo:qo + LQ],
                                    start=(i == 0), stop=(i == n_mm - 1))
                                i += 1
                    nc.scalar.activation(expT[kb][:kr, :], scp[:kr, :],
                                         ActF.Exp, bias=nb[:kr, :])
                ot = postp.tile([128, 2 * DV], F32, tag=f"ot{su}",
                                name=f"ot{su}", bufs=1)
                for qb in range(LQ // 128):
                    av = av_psp.tile([128, DV + 1], F32, tag="av", name="av")
                    for kb in range(KB):
                        kr = min(128, K - kb * 128)
                        nc.tensor.matmul(
                            av[:, :],
                            expT[kb][:kr, qb * 128:(qb + 1) * 128],
                            vx_t[su][:kr, kb * (DV + 1):
                                     kb * (DV + 1) + DV + 1],
                            start=(kb == 0), stop=(kb == KB - 1))
                    rec = postp.tile([128, 1], F32, tag="rec", name="rec")
                    nc.vector.reciprocal(rec[:], av[:, DV:DV + 1])
                    nc.scalar.activation(ot[:, qb * DV:(qb + 1) * DV],
                                         av[:, 0:DV], ActF.Copy,
                                         scale=rec[:])
                nc.sync.dma_start(out_ext, ot[:])
    nc.compile()
    return nc


_CACHE = {}


def _get_graph(slot_ks):
    key = tuple(slot_ks)
    if key not in _CACHE:
        _CACHE[key] = _build(slot_ks)
    return _CACHE[key]


def _build_in_maps(queries, keys, values, valid_lens, W_q, W_k, w_v, slots):
    # packed weights: block (d, hb) major, wq then wk
    wqk = np.empty((128, 1024), np.float16)
    for d in range(2):
        for hb in range(2):
            wqk[:, d * 256 + hb * 128:d * 256 + hb * 128 + 128] = \
                W_q[d * 128:(d + 1) * 128,
                    hb * 128:(hb + 1) * 128].astype(np.float16)
            wqk[:, 512 + d * 256 + hb * 128:512 + d * 256 + hb * 128 + 128] \
                = W_k[d * 128:(d + 1) * 128,
                      hb * 128:(hb + 1) * 128].astype(np.float16)
    wvs = np.ascontiguousarray(
        (COEF[0] * w_v).astype(np.float32).reshape(2, 128).T)
    in_maps = [{"Wqk": wqk, "wv": wvs} for _ in range(N_CORES)]
    for su, (K, grp) in enumerate(slots):
        C = LQ + K
        KB = (K + 127) // 128
        for c, b in enumerate(grp):
            vl = int(valid_lens[b])
            qT = queries[b].T.astype(np.float16)          # [D, LQ]
            kT = keys[b, :K, :].T.astype(np.float16)      # [D, K]
            qk = np.empty((128, 2 * C), np.float16)
            for d in range(2):
                qk[:, d * C:d * C + LQ] = qT[d * 128:(d + 1) * 128]
                qk[:, d * C + LQ:(d + 1) * C] = kT[d * 128:(d + 1) * 128]
            in_maps[c][f"qkT{su}"] = qk
            vpad = np.zeros((KB * 128, DV + 1), np.float16)
            vpad[:vl, :DV] = values[b, :vl, :].astype(np.float16)
            vpad[:vl, DV] = 1.0
            in_maps[c][f"vx{su}"] = np.ascontiguousarray(
                vpad.reshape(KB, 128, DV + 1).transpose(1, 0, 2)
                .reshape(128, KB * (DV + 1)))
    return in_maps


def kernel(queries, keys, values, valid_lens, W_q, W_k, w_v):
    queries = np.asarray(queries, dtype=np.float32)
    keys = np.asarray(keys, dtype=np.float32)
    values = np.asarray(values, dtype=np.float32)
    valid_lens = np.asarray(valid_lens)
    W_q = np.asarray(W_q, dtype=np.float32)
    W_k = np.asarray(W_k, dtype=np.float32)
    w_v = np.asarray(w_v, dtype=np.float32)

    slots = _plan(valid_lens)
    nc = _get_graph([K for (K, _) in slots])
    in_maps = _build_in_maps(queries, keys, values, valid_lens,
                             W_q, W_k, w_v, slots)
    res = run_bass_kernel_spmd(nc, in_maps, list(range(N_CORES)))

    out = np.empty((B, LQ, DV), np.float32)
    for su, (K, grp) in enumerate(slots):
        for c, b in enumerate(grp):
            o = res.results[c][f"out{su}"]        # [128, 2*DV]
            out[b] = o.reshape(128, 2, DV).transpose(1, 0, 2).reshape(LQ, DV)
    return out
